# revision 41
# baseline (speedup 1.0000x reference)
"""MultiHeadAttention Trainium2 kernel (8-core SPMD, zero collectives).

Problem: B=4, T=2048, E=1024, H=16, D=64 multi-head self-attention
(torch-style Linear projections, softmax over keys, output projection).

Sharding (token-split): core c handles batch b=c//2 and query-token half
qh=c%2 (tokens qh*1024..qh*1024+1024) for ALL 16 heads.  K/V projections
cover the full 2048 keys (duplicated across the pair, +~55us PE), but
every core computes its 1024 output rows completely locally -- no
AllGather, no cross-core dependency of any kind.  The paired-AllGather
variant measured ~0.9ms/exec slower at execution granularity: each
collective re-synchronizes all 8 cores, so chained executions absorb the
full cross-core launch skew four times per call.

Device pipeline per core (storage bf16, all accumulation fp32), emitted
as an explicitly software-pipelined "weave" over 32 units
(16 heads x 2 query blocks of 512) -- engines execute their instruction
streams in order, so overlap must exist at emission time:
  - per unit: 8 kc-pair score groups (K=64 matmuls, scores^T layout),
    each followed by one exp ACT [128,1024] (scale=1/8 fused, no max
    subtraction -- logits are bounded), interleaved with the previous
    unit's PV matmuls and one filler chunk (q/k projection or output
    projection work) drawn from a queue,
  - PV lhsT = [v_h | 1]: psum row 64 accumulates the softmax denominator
    for free; the PV psum tile is evacuated to SBUF immediately (a
    long-latency PSUM slot release stalls the next units' PV on HW),
    then the denominator hops to partition 0 via a tiny SBUF DMA, is
    broadcast on the idle GPSIMD engine (HW-correct from base 0 only),
    and DVE reciprocal + multiply normalizes off the copy,
  - odd heads' outputs are partition-shifted 0:64 -> 64:128 with a
    small SBUF->SBUF DMA so outT keeps a feature-major layout,
  - per query block: output projection over all 8 feature chunks of
    outT, straight from SBUF (the block's 16 heads are all local).
"""

import os
import sys
from contextlib import ExitStack

import numpy as np
import ml_dtypes

for _p in ("/opt/trn_rl_repo", "/root/.axon_site/_ro/trn_rl_repo"):
    if os.path.isdir(_p) and _p not in sys.path:
        sys.path.insert(0, _p)

import concourse.bass as bass  # noqa: E402,F401
from concourse import bacc  # noqa: E402
import concourse.tile as tile  # noqa: E402
from concourse import mybir  # noqa: E402
from concourse.bass_utils import run_bass_kernel_spmd  # noqa: E402

# ---- problem constants (hardcoded; kernel.py must be self-contained) ----
B, T, E, H, D = 4, 2048, 1024, 16, 64
P = 128
NCORES = 8
QT = T // 2          # 1024 own query tokens per core
FCH = E // P         # 8 feature chunks (head pairs)
EC = E // P          # 8 e-chunks (contraction chunks for projections)
KC = T // P          # 16 key-token chunks
QB = QT // 512       # 2 query blocks per core
TC = QT // P         # 8 output token chunks per core

BF = mybir.dt.bfloat16
F32 = mybir.dt.float32
F32R = mybir.dt.float32r
AF = mybir.ActivationFunctionType
ALU = mybir.AluOpType

SECTIONS = []        # (name, first_instruction_index) markers for profiling
USE_TILE_POS = os.environ.get("KERNEL_NO_TILEPOS", "0") != "1"
REPEAT = int(os.environ.get("KERNEL_REPEAT", "1"))
# timing-only ablations (results are WRONG when set; never set for grading)
NULLBODY = os.environ.get("KERNEL_NULLBODY", "0") == "1"
HALF_EXP = os.environ.get("KERNEL_HALF_EXP", "0") == "1"
HALF_PV = os.environ.get("KERNEL_HALF_PV", "0") == "1"
HALF_SCORES = os.environ.get("KERNEL_HALF_SCORES", "0") == "1"


def build_program():
    nc = bacc.Bacc("TRN2", target_bir_lowering=False, debug=False,
                   num_devices=NCORES)

    def mark(name):
        SECTIONS.append((name, len(nc.inst_map)))

    xt_d = nc.dram_tensor("xt", [EC, P, T], BF, kind="ExternalInput").ap()
    # wq/wk are chunk-major [FCH, P, EC, P]: one feature chunk's weights
    # are 2KB-per-partition contiguous, so each dma_w is a single clean
    # DMA instead of 8 strided 256B-per-partition ones (those pay the
    # sub-512B read-modify-write penalty and 128 descriptors each)
    wqt_d = nc.dram_tensor("wqt", [FCH, P, EC, P], BF,
                           kind="ExternalInput").ap()
    wkt_d = nc.dram_tensor("wkt", [FCH, P, EC, P], BF,
                           kind="ExternalInput").ap()
    wvt_d = nc.dram_tensor("wvt", [EC, P, E], BF, kind="ExternalInput").ap()
    wot_d = nc.dram_tensor("wot", [EC, P, E], BF, kind="ExternalInput").ap()
    bq_d = nc.dram_tensor("bq", [FCH, P], F32, kind="ExternalInput").ap()
    bk_d = nc.dram_tensor("bk", [FCH, P], F32, kind="ExternalInput").ap()
    bvb_d = nc.dram_tensor("bvb", [P, E], BF, kind="ExternalInput").ap()
    bob_d = nc.dram_tensor("bob", [P, E], BF, kind="ExternalInput").ap()
    ones_d = nc.dram_tensor("ones", [P, 64], F32R, kind="ExternalInput").ap()
    out_d = nc.dram_tensor("out", [TC, P, E], F32,
                           kind="ExternalOutput").ap()

    with tile.TileContext(nc) as tc, ExitStack() as ctx:
        persist = ctx.enter_context(tc.tile_pool(name="persist", bufs=1))
        wq_pool = ctx.enter_context(tc.tile_pool(name="wq", bufs=2))
        wv_pool = ctx.enter_context(tc.tile_pool(name="wv", bufs=1))
        small = ctx.enter_context(tc.tile_pool(name="small", bufs=2))
        otmp_pool = ctx.enter_context(tc.tile_pool(name="otmp", bufs=2))
        fin_pool = ctx.enter_context(tc.tile_pool(name="finp", bufs=2))
        psc = ctx.enter_context(tc.tile_pool(name="psc", bufs=3, space="PSUM"))
        ppv = ctx.enter_context(tc.tile_pool(name="ppv", bufs=2, space="PSUM"))

        def sc_slot():
            return psc.tile([P, 2, 512], F32, tag="sc", name="sc")

        # ---------------- persistent SBUF tensors ----------------
        xt_sb = persist.tile([P, EC, T], BF, tag="xt")          # 32K
        bq_sb = persist.tile([P, FCH], F32, tag="bq")
        bk_sb = persist.tile([P, FCH], F32, tag="bk")
        bvb_sb = persist.tile([P, E], BF, tag="bvb")            # 2K
        bob_sb = persist.tile([P, E], BF, tag="bob")            # 2K
        wot_sb = persist.tile([P, EC, E], BF, tag="wot")        # 16K
        qt_sb = persist.tile([P, FCH, QT], BF, tag="qt")        # 16K
        kt_sb = persist.tile([P, FCH, T], BF, tag="kt")         # 32K
        vaug = persist.tile([P, KC, H * 65], BF, tag="vaug")    # 32.5K
        outT = persist.tile([P, FCH, QT], BF, tag="outT")       # 16K
        ones_sb = persist.tile([P, 64], F32R, tag="ones")
        eh2 = [persist.tile([P, KC, 512], BF, tag="eh0", name="eh0"),
               persist.tile([P, KC, 512], BF, tag="eh1", name="eh1")]

        mark('in_dma')
        # ---------------- input DMAs ----------------
        # feature-chunk-0 q/k weights FIRST: the first projection matmul
        # needs wqf[ec0..3] + xt[ec0..3]; queueing the small weight DMAs
        # ahead of the 4MB xt stream starts PE ~8us earlier
        wqf0 = wq_pool.tile([P, EC, P], BF, tag="wqf", name="wqf")
        wkf0 = wq_pool.tile([P, EC, P], BF, tag="wkf", name="wkf")
        nc.sync.dma_start(wqf0[:], wqt_d[0])
        nc.sync.dma_start(wkf0[:], wkt_d[0])
        # own-token halves first: the first q-projection matmuls read only
        # xt[:, ec, 0:1024], so they unblock after half the xt bytes
        for ec in range(EC):
            nc.sync.dma_start(xt_sb[:, ec, 0:QT], xt_d[ec, :, 0:QT])
        for ec in range(EC):
            nc.sync.dma_start(xt_sb[:, ec, QT:T], xt_d[ec, :, QT:T])
        nc.sync.dma_start(bq_sb[:], bq_d.rearrange("f p -> p f"))
        nc.sync.dma_start(bk_sb[:], bk_d.rearrange("f p -> p f"))
        nc.sync.dma_start(bvb_sb[:], bvb_d)
        nc.sync.dma_start(ones_sb[:], ones_d)
        vaug_h = vaug.rearrange("p k (h c) -> p k h c", c=65)
        for h in range(H):
            nc.gpsimd.memset(vaug_h[:, :, h, 64:65], 1.0)
        if HALF_EXP:
            # timing probe: unwritten eh halves must still be finite
            nc.gpsimd.memset(eh2[0][:], 0.5)
            nc.gpsimd.memset(eh2[1][:], 0.5)

        # units: query-block major; all 16 heads of block qb complete
        # before block qb+1, so outproj(qb) can drain during qb+1
        UNITS = [(h, qb) for qb in range(QB) for h in range(H)]

        filler = []

        def drain_filler(n=1):
            for _ in range(n):
                if not filler:
                    return
                filler.pop(0)()

        def queue_qk(fc, pre=None):
            """qT/kT projection for feature chunk fc as fillers."""
            fs = slice(fc * P, (fc + 1) * P)
            box = {}

            if pre is not None:
                box["wqf"], box["wkf"] = pre
            else:
                def dma_w():
                    box["wqf"] = wq_pool.tile([P, EC, P], BF, tag="wqf",
                                              name="wqf")
                    box["wkf"] = wq_pool.tile([P, EC, P], BF, tag="wkf",
                                              name="wkf")
                    nc.sync.dma_start(box["wqf"][:], wqt_d[fc])
                    nc.sync.dma_start(box["wkf"][:], wkt_d[fc])
                filler.append(dma_w)

            def group(kind, tp):
                """One [P, 1024]-token projection group (q: own tokens,
                k: token half tp)."""
                w_key = "wqf" if kind == "q" else "wkf"
                ps_box = {}

                def mms(lo, hi):
                    def _f():
                        if "ps" not in ps_box:
                            ps_box["ps"] = sc_slot()
                        ps = ps_box["ps"]
                        for i in range(2):
                            # q: own tokens are xt columns 0:1024
                            tb = i if kind == "q" else 2 * tp + i
                            for ec in range(lo, hi):
                                nc.tensor.matmul(
                                    ps[:, i, :], lhsT=box[w_key][:, ec, :],
                                    rhs=xt_sb[:, ec, tb * 512:(tb + 1) * 512],
                                    start=(ec == 0), stop=(ec == EC - 1),
                                )
                    return _f
                filler.append(mms(0, 4))
                filler.append(mms(4, 8))

                def evac():
                    if kind == "q":
                        dest = qt_sb[:, fc, :]
                        bias = bq_sb
                    else:
                        dest = kt_sb[:, fc, tp * 1024:(tp + 1) * 1024]
                        bias = bk_sb
                    nc.vector.tensor_scalar_add(
                        dest,
                        ps_box["ps"].rearrange("p a b -> p (a b)"),
                        bias[:, fc: fc + 1],
                    )
                filler.append(evac)
            # q: own 1024 tokens = global token half QH; expressed as the
            # token pair (2*QH, 2*QH+1) -- but QH is per-core!  q tokens are
            # addressed via qoff below, uniform across cores.
            group("q", None)
            for tp in range(T // 1024):
                group("k", tp)

        # per-core own-token offset: SPMD-uniform program, per-core data --
        # host passes xt with the core's own tokens FIRST (rotated), so
        # own q tokens are always xt[:, :, 0:1024] and keys cover all T.
        # (See _prep_core_inputs: xt columns are rotated so that global
        # token (qh*1024 + t) sits at column t; kt/v cover the full ring.)

        def queue_outproj(qb):
            """final[t, all E cols] for the 4 token chunks of block qb."""
            for tcl in range(4):
                tc_ = qb * 4 + tcl
                ps_box = {}

                def mms(lo, hi, tc_=tc_, ps_box=ps_box):
                    def _f():
                        if "ps" not in ps_box:
                            ps_box["ps"] = sc_slot()
                        ps = ps_box["ps"]
                        for fc in range(lo, hi):
                            for j in range(2):
                                nc.tensor.matmul(
                                    ps[:, j, :],
                                    lhsT=outT[:, fc, tc_ * P:(tc_ + 1) * P],
                                    rhs=wot_sb[:, fc, j * 512:(j + 1) * 512],
                                    start=(fc == 0), stop=(fc == EC - 1),
                                )
                    return _f
                filler.append(mms(0, 4))
                filler.append(mms(4, 8))

                def evac(tc_=tc_, ps_box=ps_box):
                    # two pipelined [P,512] pieces: DMA of piece 0 overlaps
                    # the DVE add of piece 1 (same SBUF as one [P,1024] buf)
                    for j in range(2):
                        js = slice(j * 512, (j + 1) * 512)
                        fin = fin_pool.tile([P, 512], F32, tag="fin",
                                            name="fin")
                        nc.vector.tensor_tensor(
                            fin[:], ps_box["ps"][:, j, :], bob_sb[:, js],
                            ALU.add)
                        nc.sync.dma_start(out_d[tc_][:, js], fin[:])
                filler.append(evac)

        pv_state = {}

        def pv_mms(ui, kc):
            if HALF_PV and kc % 2 == 1:
                return
            h, qb = UNITS[ui]
            nc.tensor.matmul(
                pv_state[ui]["po"][0:65, :], lhsT=vaug_h[:, kc, h, :],
                rhs=eh2[ui % 2][:, kc, :],
                start=(kc == 0), stop=(kc == KC - 1 - (1 if HALF_PV else 0)),
            )

        def pv_finish(ui):
            h, qb = UNITS[ui]
            hp, par = h // 2, h % 2
            qs = slice(qb * 512, (qb + 1) * 512)
            po = pv_state.pop(ui)["po"]
            srb = small.tile([P, 512], F32R, tag="srb", name="srb")
            # hop the denominator (po partition 64) to partition 0 with a
            # tiny SBUF->SBUF DMA, then broadcast it on the idle GPSIMD
            # engine -- partition_broadcast is HW-correct from base 0 only
            # (base-64 sources return garbage).  Replaces a 512-cycle
            # ones-matmul + a PSUM slot per unit on PE.
            nc.vector.tensor_copy(srb[64:65, :], po[64:65, :])
            # evacuate po to SBUF immediately: the PSUM slot's release
            # must not chain through the DMA-hop -> Pool broadcast ->
            # reciprocal latency below, or PV of unit ui+2 stalls on the
            # ppv rotation (long-latency PSUM holders cost ~10x their
            # nominal time on HW)
            pcp = otmp_pool.tile([P, 512], BF, tag="ot", name="ot")
            nc.vector.tensor_copy(pcp[0:64, :], po[0:64, :])
            nc.sync.dma_start(srb[0:1, :], srb[64:65, :])
            nc.gpsimd.partition_broadcast(srb[0:64, :], srb[0:1, :])
            with nc.allow_low_precision(
                    reason="float32r is bit-identical fp32 storage"):
                nc.vector.reciprocal(srb[0:64, :], srb[0:64, :])
            if par == 0:
                nc.vector.tensor_tensor(outT[0:64, hp, qs], pcp[0:64, :],
                                        srb[0:64, :], ALU.mult)
            else:
                nc.vector.tensor_tensor(pcp[0:64, :], pcp[0:64, :],
                                        srb[0:64, :], ALU.mult)
                nc.sync.dma_start(outT[64:128, hp, qs], pcp[0:64, :])

        def weave_unit(ui, do_scores=True):
            """Emit unit ui's scores+exp interleaved with unit ui-1's PV
            and filler chunks."""
            if not do_scores:
                prev, ui = ui, None
            else:
                prev = ui - 1 if ui > 0 else None
            if prev is not None and prev not in pv_state:
                prev = None
            if ui is not None:
                h, qb = UNITS[ui]
                hp, par = h // 2, h % 2
                qs = slice(qb * 512, (qb + 1) * 512)
                rows = slice(0, 64) if par == 0 else slice(64, 128)
                tp = (dict(tile_position=(0, 0)) if par == 0 else
                      dict(tile_position=(64, 0))) if USE_TILE_POS else {}
                eh = eh2[ui % 2]
            for g in range(KC // 2):
                if ui is not None:
                    ps2 = sc_slot()
                    for i in range(1 if HALF_SCORES else 2):
                        kc = 2 * g + i
                        kslc = slice(kc * P, (kc + 1) * P)
                        nc.tensor.matmul(
                            ps2[:, i, :], lhsT=kt_sb[rows, hp, kslc],
                            rhs=qt_sb[rows, hp, qs],
                            start=True, stop=True, **tp,
                        )
                    if HALF_SCORES:
                        nc.tensor.matmul(
                            ps2[:, 1, 0:1], lhsT=kt_sb[rows, hp, 0:P],
                            rhs=qt_sb[rows, hp, qb * 512: qb * 512 + 1],
                            start=True, stop=True, **tp,
                        )
                    if HALF_EXP:
                        nc.scalar.activation(eh[:, 2 * g: 2 * g + 1, :],
                                             ps2[:, 0:1, :], AF.Exp,
                                             scale=0.125)
                    else:
                        nc.scalar.activation(eh[:, 2 * g: 2 * g + 2, :],
                                             ps2[:], AF.Exp, scale=0.125)
                if prev is not None:
                    pv_mms(prev, 2 * g)
                    pv_mms(prev, 2 * g + 1)
                drain_filler(1)
            if prev is not None:
                pv_finish(prev)
            if ui is not None:
                pv_state[ui] = {
                    "po": ppv.tile([P, 512], F32, tag="po", name="po")}

        def emit_body():
            if NULLBODY:
                # timing ablation: same I/O signature, trivial body
                zf = fin_pool.tile([P, E], F32, tag="fin", name="fin")
                nc.gpsimd.memset(zf[:], 0.0)
                for tc_ in range(TC):
                    nc.sync.dma_start(out_d[tc_], zf[:])
                return
            # bootstrap: first feature chunk's projections + first unit's
            # scores, then the v projection (ACT drains exp(0,0) meanwhile)
            queue_qk(0, pre=(wqf0, wkf0))
            drain_filler(100)
            weave_unit(0)

            queue_qk(1)

            mark('v_proj')
            # ------------- v projection (token-major, +bv) -------------
            # two half-feature passes so the weight tile stays at 8K/part
            bvb_v = bvb_sb.rearrange("p (h d) -> p h d", d=D)
            for half in range(2):
                hs = slice(half * 512, (half + 1) * 512)
                wvh = wv_pool.tile([P, EC, 512], BF, tag="wvh", name="wvh")
                for ec in range(EC):
                    nc.sync.dma_start(wvh[:, ec, :], wvt_d[ec, :, hs])
                for kc in range(KC):
                    ps = sc_slot()
                    for ec in range(EC):
                        nc.tensor.matmul(
                            ps[:, 0, :],
                            lhsT=xt_sb[:, ec, kc * P:(kc + 1) * P],
                            rhs=wvh[:, ec, :],
                            start=(ec == 0), stop=(ec == EC - 1),
                        )
                    nc.vector.tensor_tensor(
                        vaug_h[:, kc, half * 8:(half + 1) * 8, 0:64],
                        ps.rearrange("p a (h d) -> p (a h) d", d=D)[:, 0:8, :],
                        bvb_v[:, half * 8:(half + 1) * 8, :],
                        ALU.add,
                    )
                    drain_filler(1)

            mark('attention')
            for ec in range(EC):
                nc.sync.dma_start(wot_sb[:, ec, :], wot_d[ec])
            nc.sync.dma_start(bob_sb[:], bob_d)

            for ui in range(1, len(UNITS)):
                if ui in (1, 3, 5, 7, 9, 11):   # qk(fc) before unit 2*fc
                    queue_qk(ui // 2 + 2)
                if ui == 17:
                    # 1 unit into block 1: outT(block 0) is complete
                    queue_outproj(0)
                weave_unit(ui)
            weave_unit(len(UNITS) - 1, do_scores=False)
            drain_filler(100)
            queue_outproj(QB - 1)
            drain_filler(100)

        for _rep in range(REPEAT):
            emit_body()

        mark('tail')
    nc.compile()
    return nc


_NC = None


def _get_nc():
    global _NC
    if _NC is None:
        _NC = build_program()
    return _NC


def _prep_core_inputs(x, Wq, bq, Wk, bk, Wv, bv, Wo, bo):
    """Build the 8 per-core input dicts (host-side sharding).

    Core c = (batch c//2, query-half c%2).  xt's token columns are
    ROTATED so the core's own 1024 query tokens sit first: column t holds
    global token (qh*1024 + t) mod 2048.  q reads columns 0:1024; k/v
    cover all 2048 (order irrelevant -- softmax sums over keys); the
    rotation is identical for every tensor derived from xt, so scores,
    PV, and the output projection all see consistent key ordering.
    """
    bf = ml_dtypes.bfloat16
    x = np.asarray(x, dtype=np.float32)
    Wq, Wk, Wv, Wo = (np.asarray(a, np.float32) for a in (Wq, Wk, Wv, Wo))
    bq, bk, bv, bo = (np.asarray(a, np.float32) for a in (bq, bk, bv, bo))
    ones_a = np.ones((P, 64), np.float32)

    # chunk-major [FCH, P, EC, P]: wqt[fc, p, ec, j] = Wq.T[ec*P+p, fc*P+j]
    wqt = np.ascontiguousarray(
        Wq.T.astype(bf).reshape(EC, P, FCH, P).transpose(2, 1, 0, 3))
    wkt = np.ascontiguousarray(
        Wk.T.astype(bf).reshape(EC, P, FCH, P).transpose(2, 1, 0, 3))
    wvt = np.ascontiguousarray(Wv.T).astype(bf).reshape(EC, P, E)
    wot = np.ascontiguousarray(Wo.T).astype(bf).reshape(EC, P, E)
    bq_a = np.ascontiguousarray(bq).reshape(FCH, P)
    bk_a = np.ascontiguousarray(bk).reshape(FCH, P)
    bvb = np.ascontiguousarray(
        np.broadcast_to(bv[None, :], (P, E))).astype(bf)
    bob = np.ascontiguousarray(
        np.broadcast_to(bo[None, :], (P, E))).astype(bf)

    in_maps = []
    for c in range(NCORES):
        b, qh = c // 2, c % 2
        xb = x[b]
        if qh:
            xb = np.concatenate([xb[QT:], xb[:QT]], axis=0)
        xt = np.ascontiguousarray(xb.T).astype(bf).reshape(EC, P, T)
        in_maps.append({
            "xt": xt, "wqt": wqt, "wkt": wkt, "wvt": wvt,
            "wot": wot, "bq": bq_a, "bk": bk_a,
            "bvb": bvb, "bob": bob, "ones": ones_a,
        })
    return in_maps


def kernel(x, Wq, bq, Wk, bk, Wv, bv, Wo, bo):
    nc = _get_nc()
    in_maps = _prep_core_inputs(x, Wq, bq, Wk, bk, Wv, bv, Wo, bo)
    res = run_bass_kernel_spmd(nc, in_maps, list(range(NCORES)))
    out = np.empty((B, T, E), np.float32)
    for c in range(NCORES):
        b, qh = c // 2, c % 2
        out[b, qh * QT:(qh + 1) * QT, :] = res.results[c]["out"].reshape(QT, E)
    return out


# revision 42
# speedup vs baseline: 1.1352x; 1.1352x over previous
"""MultiHeadAttention Trainium2 kernel (8-core SPMD, zero collectives).

Problem: B=4, T=2048, E=1024, H=16, D=64 multi-head self-attention
(torch-style Linear projections, softmax over keys, output projection).

Sharding (token-split): core c handles batch b=c//2 and query-token half
qh=c%2 (tokens qh*1024..qh*1024+1024) for ALL 16 heads.  K/V projections
cover the full 2048 keys (duplicated across the pair, +~55us PE), but
every core computes its 1024 output rows completely locally -- no
AllGather, no cross-core dependency of any kind.  The paired-AllGather
variant measured ~0.9ms/exec slower at execution granularity: each
collective re-synchronizes all 8 cores, so chained executions absorb the
full cross-core launch skew four times per call.

Device pipeline per core (storage bf16, all accumulation fp32), emitted
as an explicitly software-pipelined "weave" over 32 units
(16 heads x 2 query blocks of 512) -- engines execute their instruction
streams in order, so overlap must exist at emission time:
  - per unit: 8 kc-pair score groups (K=64 matmuls, scores^T layout),
    each followed by one exp ACT [128,1024] (scale=1/8 fused, no max
    subtraction -- logits are bounded), interleaved with the previous
    unit's PV matmuls and one filler chunk (q/k projection or output
    projection work) drawn from a queue,
  - PV lhsT = [v_h | 1]: psum row 64 accumulates the softmax denominator
    for free; the PV psum tile is evacuated to SBUF immediately (a
    long-latency PSUM slot release stalls the next units' PV on HW),
    then the denominator hops to partition 0 via a tiny SBUF DMA, is
    broadcast on the idle GPSIMD engine (HW-correct from base 0 only),
    and DVE reciprocal + multiply normalizes off the copy,
  - odd heads' outputs are partition-shifted 0:64 -> 64:128 with a
    small SBUF->SBUF DMA so outT keeps a feature-major layout,
  - per query block: output projection over all 8 feature chunks of
    outT, straight from SBUF (the block's 16 heads are all local).
"""

import os
import sys
from contextlib import ExitStack

import numpy as np
import ml_dtypes

for _p in ("/opt/trn_rl_repo", "/root/.axon_site/_ro/trn_rl_repo"):
    if os.path.isdir(_p) and _p not in sys.path:
        sys.path.insert(0, _p)

import concourse.bass as bass  # noqa: E402,F401
from concourse import bacc  # noqa: E402
import concourse.tile as tile  # noqa: E402
from concourse import mybir  # noqa: E402
from concourse.bass_utils import run_bass_kernel_spmd  # noqa: E402

# ---- problem constants (hardcoded; kernel.py must be self-contained) ----
B, T, E, H, D = 4, 2048, 1024, 16, 64
P = 128
NCORES = 8
QT = T // 2          # 1024 own query tokens per core
FCH = E // P         # 8 feature chunks (head pairs)
EC = E // P          # 8 e-chunks (contraction chunks for projections)
KC = T // P          # 16 key-token chunks
QB = QT // 512       # 2 query blocks per core
TC = QT // P         # 8 output token chunks per core

BF = mybir.dt.bfloat16
F32 = mybir.dt.float32
F32R = mybir.dt.float32r
AF = mybir.ActivationFunctionType
ALU = mybir.AluOpType

SECTIONS = []        # (name, first_instruction_index) markers for profiling
USE_TILE_POS = os.environ.get("KERNEL_NO_TILEPOS", "0") != "1"
REPEAT = int(os.environ.get("KERNEL_REPEAT", "1"))
# timing-only ablations (results are WRONG when set; never set for grading)
NULLBODY = os.environ.get("KERNEL_NULLBODY", "0") == "1"
HALF_EXP = os.environ.get("KERNEL_HALF_EXP", "0") == "1"
HALF_PV = os.environ.get("KERNEL_HALF_PV", "0") == "1"
HALF_SCORES = os.environ.get("KERNEL_HALF_SCORES", "0") == "1"


def build_program():
    nc = bacc.Bacc("TRN2", target_bir_lowering=False, debug=False,
                   num_devices=NCORES)

    def mark(name):
        SECTIONS.append((name, len(nc.inst_map)))

    xt_d = nc.dram_tensor("xt", [EC, P, T], BF, kind="ExternalInput").ap()
    # wq/wk are chunk-major [FCH, P, EC, P]: one feature chunk's weights
    # are 2KB-per-partition contiguous, so each dma_w is a single clean
    # DMA instead of 8 strided 256B-per-partition ones (those pay the
    # sub-512B read-modify-write penalty and 128 descriptors each)
    wqt_d = nc.dram_tensor("wqt", [FCH, P, EC, P], BF,
                           kind="ExternalInput").ap()
    wkt_d = nc.dram_tensor("wkt", [FCH, P, EC, P], BF,
                           kind="ExternalInput").ap()
    wvt_d = nc.dram_tensor("wvt", [EC, P, E], BF, kind="ExternalInput").ap()
    wot_d = nc.dram_tensor("wot", [EC, P, E], BF, kind="ExternalInput").ap()
    bq_d = nc.dram_tensor("bq", [FCH, P], F32, kind="ExternalInput").ap()
    bk_d = nc.dram_tensor("bk", [FCH, P], F32, kind="ExternalInput").ap()
    bvb_d = nc.dram_tensor("bvb", [P, E], BF, kind="ExternalInput").ap()
    bob_d = nc.dram_tensor("bob", [P, E], BF, kind="ExternalInput").ap()
    ones_d = nc.dram_tensor("ones", [P, 64], F32R, kind="ExternalInput").ap()
    out_d = nc.dram_tensor("out", [TC, P, E], F32,
                           kind="ExternalOutput").ap()

    with tile.TileContext(nc) as tc, ExitStack() as ctx:
        persist = ctx.enter_context(tc.tile_pool(name="persist", bufs=1))
        wq_pool = ctx.enter_context(tc.tile_pool(name="wq", bufs=2))
        wv_pool = ctx.enter_context(tc.tile_pool(name="wv", bufs=1))
        small = ctx.enter_context(tc.tile_pool(name="small", bufs=2))
        otmp_pool = ctx.enter_context(tc.tile_pool(name="otmp", bufs=2))
        fin_pool = ctx.enter_context(tc.tile_pool(name="finp", bufs=2))
        psc = ctx.enter_context(tc.tile_pool(name="psc", bufs=3, space="PSUM"))
        ppv = ctx.enter_context(tc.tile_pool(name="ppv", bufs=2, space="PSUM"))

        def sc_slot():
            return psc.tile([P, 2, 512], F32, tag="sc", name="sc")

        # ---------------- persistent SBUF tensors ----------------
        xt_sb = persist.tile([P, EC, T], BF, tag="xt")          # 32K
        bq_sb = persist.tile([P, FCH], F32, tag="bq")
        bk_sb = persist.tile([P, FCH], F32, tag="bk")
        bvb_sb = persist.tile([P, E], BF, tag="bvb")            # 2K
        bob_sb = persist.tile([P, E], BF, tag="bob")            # 2K
        wot_sb = persist.tile([P, EC, E], BF, tag="wot")        # 16K
        qt_sb = persist.tile([P, FCH, QT], BF, tag="qt")        # 16K
        kt_sb = persist.tile([P, FCH, T], BF, tag="kt")         # 32K
        vaug = persist.tile([P, KC, H * 65], BF, tag="vaug")    # 32.5K
        outT = persist.tile([P, FCH, QT], BF, tag="outT")       # 16K
        ones_sb = persist.tile([P, 64], F32R, tag="ones")
        eh2 = [persist.tile([P, KC, 512], BF, tag="eh0", name="eh0"),
               persist.tile([P, KC, 512], BF, tag="eh1", name="eh1")]

        mark('in_dma')
        # ---------------- input DMAs ----------------
        # feature-chunk-0 q/k weights FIRST: the first projection matmul
        # needs wqf[ec0..3] + xt[ec0..3]; queueing the small weight DMAs
        # ahead of the 4MB xt stream starts PE ~8us earlier
        wqf0 = wq_pool.tile([P, EC, P], BF, tag="wqf", name="wqf")
        wkf0 = wq_pool.tile([P, EC, P], BF, tag="wkf", name="wkf")
        nc.sync.dma_start(wqf0[:], wqt_d[0])
        nc.sync.dma_start(wkf0[:], wkt_d[0])
        # own-token halves first: the first q-projection matmuls read only
        # xt[:, ec, 0:1024], so they unblock after half the xt bytes
        for ec in range(EC):
            nc.sync.dma_start(xt_sb[:, ec, 0:QT], xt_d[ec, :, 0:QT])
        for ec in range(EC):
            nc.sync.dma_start(xt_sb[:, ec, QT:T], xt_d[ec, :, QT:T])
        nc.sync.dma_start(bq_sb[:], bq_d.rearrange("f p -> p f"))
        nc.sync.dma_start(bk_sb[:], bk_d.rearrange("f p -> p f"))
        nc.sync.dma_start(bvb_sb[:], bvb_d)
        nc.sync.dma_start(ones_sb[:], ones_d)
        vaug_h = vaug.rearrange("p k (h c) -> p k h c", c=65)
        for h in range(H):
            nc.gpsimd.memset(vaug_h[:, :, h, 64:65], 1.0)
        if HALF_EXP:
            # timing probe: unwritten eh halves must still be finite
            nc.gpsimd.memset(eh2[0][:], 0.5)
            nc.gpsimd.memset(eh2[1][:], 0.5)

        # units: query-block major; all 16 heads of block qb complete
        # before block qb+1, so outproj(qb) can drain during qb+1.
        # Each block ends on an EVEN-parity head (14 last, 15 swapped
        # earlier): the block-gating pv_finish then writes outT directly
        # instead of through the odd-head DMA partition-shift, taking one
        # DMA out of the outproj-release / tail critical path.
        HORDER = list(range(14)) + [15, 14]
        UNITS = [(h, qb) for qb in range(QB) for h in HORDER]

        filler = []

        def drain_filler(n=1):
            for _ in range(n):
                if not filler:
                    return
                filler.pop(0)()

        def queue_qk(fc, pre=None):
            """qT/kT projection for feature chunk fc as fillers."""
            fs = slice(fc * P, (fc + 1) * P)
            box = {}

            if pre is not None:
                box["wqf"], box["wkf"] = pre
            else:
                def dma_w():
                    box["wqf"] = wq_pool.tile([P, EC, P], BF, tag="wqf",
                                              name="wqf")
                    box["wkf"] = wq_pool.tile([P, EC, P], BF, tag="wkf",
                                              name="wkf")
                    nc.sync.dma_start(box["wqf"][:], wqt_d[fc])
                    nc.sync.dma_start(box["wkf"][:], wkt_d[fc])
                filler.append(dma_w)

            def group(kind, tp):
                """One [P, 1024]-token projection group (q: own tokens,
                k: token half tp)."""
                w_key = "wqf" if kind == "q" else "wkf"
                ps_box = {}

                def mms(lo, hi):
                    def _f():
                        if "ps" not in ps_box:
                            ps_box["ps"] = sc_slot()
                        ps = ps_box["ps"]
                        for i in range(2):
                            # q: own tokens are xt columns 0:1024
                            tb = i if kind == "q" else 2 * tp + i
                            for ec in range(lo, hi):
                                nc.tensor.matmul(
                                    ps[:, i, :], lhsT=box[w_key][:, ec, :],
                                    rhs=xt_sb[:, ec, tb * 512:(tb + 1) * 512],
                                    start=(ec == 0), stop=(ec == EC - 1),
                                )
                    return _f
                filler.append(mms(0, 4))
                filler.append(mms(4, 8))

                def evac():
                    if kind == "q":
                        dest = qt_sb[:, fc, :]
                        bias = bq_sb
                    else:
                        dest = kt_sb[:, fc, tp * 1024:(tp + 1) * 1024]
                        bias = bk_sb
                    nc.vector.tensor_scalar_add(
                        dest,
                        ps_box["ps"].rearrange("p a b -> p (a b)"),
                        bias[:, fc: fc + 1],
                    )
                filler.append(evac)
            # q: own 1024 tokens = global token half QH; expressed as the
            # token pair (2*QH, 2*QH+1) -- but QH is per-core!  q tokens are
            # addressed via qoff below, uniform across cores.
            group("q", None)
            for tp in range(T // 1024):
                group("k", tp)

        # per-core own-token offset: SPMD-uniform program, per-core data --
        # host passes xt with the core's own tokens FIRST (rotated), so
        # own q tokens are always xt[:, :, 0:1024] and keys cover all T.
        # (See _prep_core_inputs: xt columns are rotated so that global
        # token (qh*1024 + t) sits at column t; kt/v cover the full ring.)

        def queue_outproj(qb):
            """final[t, all E cols] for the 4 token chunks of block qb."""
            for tcl in range(4):
                tc_ = qb * 4 + tcl
                ps_box = {}

                def mms(lo, hi, tc_=tc_, ps_box=ps_box):
                    def _f():
                        if "ps" not in ps_box:
                            ps_box["ps"] = sc_slot()
                        ps = ps_box["ps"]
                        for fc in range(lo, hi):
                            for j in range(2):
                                nc.tensor.matmul(
                                    ps[:, j, :],
                                    lhsT=outT[:, fc, tc_ * P:(tc_ + 1) * P],
                                    rhs=wot_sb[:, fc, j * 512:(j + 1) * 512],
                                    start=(fc == 0), stop=(fc == EC - 1),
                                )
                    return _f
                filler.append(mms(0, 4))
                filler.append(mms(4, 8))

                def evac(tc_=tc_, ps_box=ps_box):
                    # two pipelined [P,512] pieces: DMA of piece 0 overlaps
                    # the DVE add of piece 1 (same SBUF as one [P,1024] buf)
                    for j in range(2):
                        js = slice(j * 512, (j + 1) * 512)
                        fin = fin_pool.tile([P, 512], F32, tag="fin",
                                            name="fin")
                        nc.vector.tensor_tensor(
                            fin[:], ps_box["ps"][:, j, :], bob_sb[:, js],
                            ALU.add)
                        nc.sync.dma_start(out_d[tc_][:, js], fin[:])
                filler.append(evac)

        pv_state = {}

        def pv_mms(ui, kc):
            if HALF_PV and kc % 2 == 1:
                return
            h, qb = UNITS[ui]
            nc.tensor.matmul(
                pv_state[ui]["po"][0:65, :], lhsT=vaug_h[:, kc, h, :],
                rhs=eh2[ui % 2][:, kc, :],
                start=(kc == 0), stop=(kc == KC - 1 - (1 if HALF_PV else 0)),
            )

        def pv_finish(ui):
            h, qb = UNITS[ui]
            hp, par = h // 2, h % 2
            qs = slice(qb * 512, (qb + 1) * 512)
            po = pv_state.pop(ui)["po"]
            srb = small.tile([P, 512], F32R, tag="srb", name="srb")
            # hop the denominator (po partition 64) to partition 0 with a
            # tiny SBUF->SBUF DMA, then broadcast it on the idle GPSIMD
            # engine -- partition_broadcast is HW-correct from base 0 only
            # (base-64 sources return garbage).  Replaces a 512-cycle
            # ones-matmul + a PSUM slot per unit on PE.
            nc.vector.tensor_copy(srb[64:65, :], po[64:65, :])
            # evacuate po to SBUF immediately: the PSUM slot's release
            # must not chain through the DMA-hop -> Pool broadcast ->
            # reciprocal latency below, or PV of unit ui+2 stalls on the
            # ppv rotation (long-latency PSUM holders cost ~10x their
            # nominal time on HW)
            pcp = otmp_pool.tile([P, 512], BF, tag="ot", name="ot")
            nc.vector.tensor_copy(pcp[0:64, :], po[0:64, :])
            nc.sync.dma_start(srb[0:1, :], srb[64:65, :])
            nc.gpsimd.partition_broadcast(srb[0:64, :], srb[0:1, :])
            with nc.allow_low_precision(
                    reason="float32r is bit-identical fp32 storage"):
                nc.vector.reciprocal(srb[0:64, :], srb[0:64, :])
            if par == 0:
                nc.vector.tensor_tensor(outT[0:64, hp, qs], pcp[0:64, :],
                                        srb[0:64, :], ALU.mult)
            else:
                nc.vector.tensor_tensor(pcp[0:64, :], pcp[0:64, :],
                                        srb[0:64, :], ALU.mult)
                nc.sync.dma_start(outT[64:128, hp, qs], pcp[0:64, :])

        def weave_unit(ui, do_scores=True):
            """Emit unit ui's scores+exp interleaved with unit ui-1's PV
            and filler chunks."""
            if not do_scores:
                prev, ui = ui, None
            else:
                prev = ui - 1 if ui > 0 else None
            if prev is not None and prev not in pv_state:
                prev = None
            if ui is not None:
                h, qb = UNITS[ui]
                hp, par = h // 2, h % 2
                qs = slice(qb * 512, (qb + 1) * 512)
                rows = slice(0, 64) if par == 0 else slice(64, 128)
                tp = (dict(tile_position=(0, 0)) if par == 0 else
                      dict(tile_position=(64, 0))) if USE_TILE_POS else {}
                eh = eh2[ui % 2]
            for g in range(KC // 2):
                if ui is not None:
                    ps2 = sc_slot()
                    for i in range(1 if HALF_SCORES else 2):
                        kc = 2 * g + i
                        kslc = slice(kc * P, (kc + 1) * P)
                        nc.tensor.matmul(
                            ps2[:, i, :], lhsT=kt_sb[rows, hp, kslc],
                            rhs=qt_sb[rows, hp, qs],
                            start=True, stop=True, **tp,
                        )
                    if HALF_SCORES:
                        nc.tensor.matmul(
                            ps2[:, 1, 0:1], lhsT=kt_sb[rows, hp, 0:P],
                            rhs=qt_sb[rows, hp, qb * 512: qb * 512 + 1],
                            start=True, stop=True, **tp,
                        )
                    if HALF_EXP:
                        nc.scalar.activation(eh[:, 2 * g: 2 * g + 1, :],
                                             ps2[:, 0:1, :], AF.Exp,
                                             scale=0.125)
                    else:
                        nc.scalar.activation(eh[:, 2 * g: 2 * g + 2, :],
                                             ps2[:], AF.Exp, scale=0.125)
                if prev is not None:
                    pv_mms(prev, 2 * g)
                    pv_mms(prev, 2 * g + 1)
                drain_filler(1)
            if prev is not None:
                pv_finish(prev)
            if ui is not None:
                pv_state[ui] = {
                    "po": ppv.tile([P, 512], F32, tag="po", name="po")}

        def emit_body():
            if NULLBODY:
                # timing ablation: same I/O signature, trivial body
                zf = fin_pool.tile([P, E], F32, tag="fin", name="fin")
                nc.gpsimd.memset(zf[:], 0.0)
                for tc_ in range(TC):
                    nc.sync.dma_start(out_d[tc_], zf[:])
                return
            # bootstrap: first feature chunk's projections + first unit's
            # scores, then the v projection (ACT drains exp(0,0) meanwhile)
            queue_qk(0, pre=(wqf0, wkf0))
            drain_filler(100)
            weave_unit(0)

            queue_qk(1)

            mark('v_proj')
            # ------------- v projection (token-major, +bv) -------------
            # two half-feature passes so the weight tile stays at 8K/part
            bvb_v = bvb_sb.rearrange("p (h d) -> p h d", d=D)
            for half in range(2):
                hs = slice(half * 512, (half + 1) * 512)
                wvh = wv_pool.tile([P, EC, 512], BF, tag="wvh", name="wvh")
                for ec in range(EC):
                    nc.sync.dma_start(wvh[:, ec, :], wvt_d[ec, :, hs])
                for kc in range(KC):
                    ps = sc_slot()
                    for ec in range(EC):
                        nc.tensor.matmul(
                            ps[:, 0, :],
                            lhsT=xt_sb[:, ec, kc * P:(kc + 1) * P],
                            rhs=wvh[:, ec, :],
                            start=(ec == 0), stop=(ec == EC - 1),
                        )
                    nc.vector.tensor_tensor(
                        vaug_h[:, kc, half * 8:(half + 1) * 8, 0:64],
                        ps.rearrange("p a (h d) -> p (a h) d", d=D)[:, 0:8, :],
                        bvb_v[:, half * 8:(half + 1) * 8, :],
                        ALU.add,
                    )
                    drain_filler(1)

            mark('attention')
            for ec in range(EC):
                nc.sync.dma_start(wot_sb[:, ec, :], wot_d[ec])
            nc.sync.dma_start(bob_sb[:], bob_d)

            for ui in range(1, len(UNITS)):
                if ui in (1, 3, 5, 7, 9, 11):   # qk(fc) before unit 2*fc
                    queue_qk(ui // 2 + 2)
                if ui == 17:
                    # 1 unit into block 1: outT(block 0) is complete
                    queue_outproj(0)
                weave_unit(ui)
            weave_unit(len(UNITS) - 1, do_scores=False)
            drain_filler(100)
            queue_outproj(QB - 1)
            drain_filler(100)

        for _rep in range(REPEAT):
            emit_body()

        mark('tail')
    nc.compile()
    return nc


_NC = None


def _get_nc():
    global _NC
    if _NC is None:
        _NC = build_program()
    return _NC


def _prep_core_inputs(x, Wq, bq, Wk, bk, Wv, bv, Wo, bo):
    """Build the 8 per-core input dicts (host-side sharding).

    Core c = (batch c//2, query-half c%2).  xt's token columns are
    ROTATED so the core's own 1024 query tokens sit first: column t holds
    global token (qh*1024 + t) mod 2048.  q reads columns 0:1024; k/v
    cover all 2048 (order irrelevant -- softmax sums over keys); the
    rotation is identical for every tensor derived from xt, so scores,
    PV, and the output projection all see consistent key ordering.
    """
    bf = ml_dtypes.bfloat16
    x = np.asarray(x, dtype=np.float32)
    Wq, Wk, Wv, Wo = (np.asarray(a, np.float32) for a in (Wq, Wk, Wv, Wo))
    bq, bk, bv, bo = (np.asarray(a, np.float32) for a in (bq, bk, bv, bo))
    ones_a = np.ones((P, 64), np.float32)

    # chunk-major [FCH, P, EC, P]: wqt[fc, p, ec, j] = Wq.T[ec*P+p, fc*P+j]
    wqt = np.ascontiguousarray(
        Wq.T.astype(bf).reshape(EC, P, FCH, P).transpose(2, 1, 0, 3))
    wkt = np.ascontiguousarray(
        Wk.T.astype(bf).reshape(EC, P, FCH, P).transpose(2, 1, 0, 3))
    wvt = np.ascontiguousarray(Wv.T).astype(bf).reshape(EC, P, E)
    wot = np.ascontiguousarray(Wo.T).astype(bf).reshape(EC, P, E)
    bq_a = np.ascontiguousarray(bq).reshape(FCH, P)
    bk_a = np.ascontiguousarray(bk).reshape(FCH, P)
    bvb = np.ascontiguousarray(
        np.broadcast_to(bv[None, :], (P, E))).astype(bf)
    bob = np.ascontiguousarray(
        np.broadcast_to(bo[None, :], (P, E))).astype(bf)

    in_maps = []
    for c in range(NCORES):
        b, qh = c // 2, c % 2
        xb = x[b]
        if qh:
            xb = np.concatenate([xb[QT:], xb[:QT]], axis=0)
        xt = np.ascontiguousarray(xb.T).astype(bf).reshape(EC, P, T)
        in_maps.append({
            "xt": xt, "wqt": wqt, "wkt": wkt, "wvt": wvt,
            "wot": wot, "bq": bq_a, "bk": bk_a,
            "bvb": bvb, "bob": bob, "ones": ones_a,
        })
    return in_maps


def kernel(x, Wq, bq, Wk, bk, Wv, bv, Wo, bo):
    nc = _get_nc()
    in_maps = _prep_core_inputs(x, Wq, bq, Wk, bk, Wv, bv, Wo, bo)
    res = run_bass_kernel_spmd(nc, in_maps, list(range(NCORES)))
    out = np.empty((B, T, E), np.float32)
    for c in range(NCORES):
        b, qh = c // 2, c % 2
        out[b, qh * QT:(qh + 1) * QT, :] = res.results[c]["out"].reshape(QT, E)
    return out
